# revision 38
# baseline (speedup 1.0000x reference)
"""Trainium2 Bass kernel for AttentiveMinkUNetDiff KNN+MLP block.

Self-contained: hardcodes shapes N=16384, M=32768, K=8, C=256, 8 cores.
Strategy: shard nodes across 8 cores; cond set replicated.

Device pipeline per core (2048 nodes, 16 tiles of 128):
  1. PE computes a per-node-ranking-equivalent of -40000*d^2 for all
     32768 cond points via an exact bf16-split integer matmul (K=19 rows).
  2. DVE max/max_index per 1024-wide PSUM super-chunk -> pool of 256
     candidates with global indices (encoded as 32768-gidx for tie order).
  3. Top-24 of pool via match_replace rounds + masked-index extraction.
  4. Exact refinement: indirect-DMA gather candidate coords, recompute d^2
     bit-exactly matching XLA's fused fma chain (Dekker emulation), pick 8.
  5. Gather cond_feats rows, inverse-distance weighted mean (weights sum to
     1 so the mean commutes with W_proj), 3-layer MLP in transposed space,
     timestep-embedding branch folded into the final bias.
"""
import math
import numpy as np
import ml_dtypes

import concourse.bass as bass
import concourse.mybir as mybir
from concourse.tile import TileContext
from concourse import bass_utils
from concourse import bacc

bf16 = ml_dtypes.bfloat16
f32 = np.float32
AF = mybir.ActivationFunctionType
OP = mybir.AluOpType

DEBUG = False
N, M, K = 16384, 32768, 8
C = 256
EMBED, HALF = 96, 48
NCORES = 8
NSHARD = N // NCORES          # 2048
NTILES = NSHARD // 128        # 16
SUP = 1024                    # distance super-chunk width (2 PSUM banks)
NSUP = M // SUP               # 32
POOL = NSUP * 8               # 256
NCAND = 24
PI = float(np.pi)


# ---------------------------------------------------------------- host prep
def _split_rows(nodes, conds):
    """Build the 19 bf16-exact contraction rows. Validated vs reference."""
    a = nodes[:, 1:4].astype(np.int64)
    b = conds[:, 1:4].astype(np.int64)
    ah, al = a >> 5, a & 31
    bh, bl = b >> 5, b & 31
    lhs, rhs = [], []
    for k in range(3):
        lhs += [1280.0 * ah[:, k], 1280.0 * ah[:, k], 40.0 * al[:, k], 40.0 * al[:, k]]
        rhs += [32.0 * bh[:, k], 1.0 * bl[:, k], 32.0 * bh[:, k], 1.0 * bl[:, k]]
    B_total = (4 * b * b - 316 * b).sum(1) + 32768
    s2, s1, s0 = B_total >> 16, (B_total >> 8) & 255, B_total & 255
    nones = -np.ones(a.shape[0])
    lhs += [nones, nones, nones]
    rhs += [s2 * 65536.0, s1 * 256.0, s0 * 1.0]
    C_i = ((10 * a + 79) ** 2).sum(1) - 32768
    c3 = np.floor(C_i / 2 ** 21).astype(np.int64)
    r = C_i - c3 * 2 ** 21
    c2, c1, c0 = r >> 13, (r >> 5) & 255, r & 31
    mones = np.ones(b.shape[0])
    lhs += [-c3 * 2097152.0, -c2 * 8192.0, -c1 * 32.0, -c0 * 1.0]
    rhs += [mones, mones, mones, mones]
    LHS = np.stack(lhs).astype(f32)   # [19, N]
    RHS = np.stack(rhs).astype(f32)   # [19, M]
    return LHS.astype(bf16), RHS.astype(bf16)


def _transform(coords, stride, voxel, mc):
    c = coords.astype(np.float32)
    batch = (c[:, :1] * f32(mc * f32(2.0))).astype(f32)
    xyz = ((c[:, 1:] + f32(stride / 2.0)).astype(f32) * f32(voxel)).astype(f32)
    return np.concatenate([batch, xyz], 1).astype(f32)


def _pack_w(w):
    """W [dout, din] -> lhsT pack [128, 4*128]: cols (ct*2+dt)*128+d for c-tile ct, d-tile dt."""
    wt = np.ascontiguousarray(w.T.astype(f32))          # [din=256, dout=256]
    p = wt.reshape(2, 128, 2, 128)                      # [ct, c, dt, d]
    p = p.transpose(1, 0, 2, 3).reshape(128, 512)       # [c, (ct,dt,d)]
    return np.ascontiguousarray(p)


_CACHE = {}


def _build_program():
    if 'nc' in _CACHE:
        return _CACHE['nc'], _CACHE['names']
    nc = bacc.Bacc("TRN2", target_bir_lowering=False, debug=False,
                   num_devices=NCORES)
    dt = mybir.dt
    T = {}

    def din(name, shape, dtype):
        T[name] = nc.dram_tensor(name, shape, dtype, kind="ExternalInput").ap()
        return T[name]

    lhsT = din('lhsT', [19, NSHARD], dt.bfloat16)
    rhs = din('rhsT', [19, M], dt.bfloat16)
    nodex = din('nodex', [128, NTILES * 3], dt.float32)      # [p, t*3+k] xyz of node
    partc = din('partc', [M, 4], dt.float32)
    feats = din('feats', [M, C], dt.float32)
    invbase = din('invbase', [128, POOL], dt.float32)
    eye = din('eye', [128, 128], dt.float32)
    wp = din('wp', [128, 512], dt.float32)
    wl1 = din('wl1', [128, 512], dt.float32)
    wl2 = din('wl2', [128, 512], dt.float32)
    bproj = din('bproj', [128, 2], dt.float32)
    bl1 = din('bl1', [128, 2], dt.float32)
    bcomb = din('bcomb', [128, 2], dt.float32)               # b_l2 + b_t2
    wt1 = din('wt1', [EMBED, EMBED], dt.float32)
    wt2 = din('wt2', [EMBED, C], dt.float32)
    bt1 = din('bt1', [EMBED, 1], dt.float32)
    freqs = din('freqs', [EMBED, 1], dt.float32)     # [freqs, freqs]
    shifts = din('shifts', [EMBED, 1], dt.float32)   # [pi]*48 + [3pi/2]*48
    tval = din('tval', [EMBED, 1], dt.float32)   # t replicated per partition
    out = nc.dram_tensor('out', [NSHARD, C], dt.float32, kind="ExternalOutput").ap()
    if DEBUG:
        dbg_psum = nc.dram_tensor('dbg_psum', [128, SUP], dt.float32,
                                  kind="ExternalOutput").ap()
        dbg_pv = nc.dram_tensor('dbg_pv', [128, POOL], dt.float32,
                                kind="ExternalOutput").ap()
        dbg_pl = nc.dram_tensor('dbg_pl', [128, POOL], dt.uint16,
                                kind="ExternalOutput").ap()
        dbg_inv = nc.dram_tensor('dbg_inv', [128, NCAND], dt.float32,
                                 kind="ExternalOutput").ap()
        dbg_g24 = nc.dram_tensor('dbg_g24', [128, NCAND], dt.uint32,
                                 kind="ExternalOutput").ap()
        dbg_d2 = nc.dram_tensor('dbg_d2', [128, NCAND], dt.float32,
                                kind="ExternalOutput").ap()
        dbg_candy = nc.dram_tensor('dbg_candy', [128, NCAND * 4], dt.float32,
                                   kind="ExternalOutput").ap()
        dbg_wn = nc.dram_tensor('dbg_wn', [128, 8], dt.float32,
                                kind="ExternalOutput").ap()
        dbg_g8 = nc.dram_tensor('dbg_g8', [128, 8], dt.uint32,
                                kind="ExternalOutput").ap()
        dbg_winy = nc.dram_tensor('dbg_winy', [128, 32], dt.float32,
                                  kind="ExternalOutput").ap()
        dbg_fT = nc.dram_tensor('dbg_fT', [128, C], dt.float32,
                                kind="ExternalOutput").ap()
        dbg_mT = nc.dram_tensor('dbg_mT', [128, C], dt.float32,
                                kind="ExternalOutput").ap()
        dbg_fin = nc.dram_tensor('dbg_fin', [128, 2], dt.float32,
                                 kind="ExternalOutput").ap()
        dbg_h1 = nc.dram_tensor('dbg_h1', [128, C], dt.float32,
                                kind="ExternalOutput").ap()
        dbg_pT = nc.dram_tensor('dbg_pT', [128, C], dt.float32,
                                kind="ExternalOutput").ap()
        dbg_osb = nc.dram_tensor('dbg_osb', [128, C], dt.float32,
                                 kind="ExternalOutput").ap()
        dbg_fbar = nc.dram_tensor('dbg_fbar', [128, C], dt.float32,
                                  kind="ExternalOutput").ap()

    with TileContext(nc) as tc, \
            tc.tile_pool(name="const", bufs=1) as cpool, \
            tc.tile_pool(name="work", bufs=2) as wpool, \
            tc.tile_pool(name="psum", bufs=2, space="PSUM") as ppool:

        # ---- load constants
        rhs_sb = cpool.tile([19, M], dt.bfloat16, tag="rhs")
        nc.sync.dma_start(out=rhs_sb[:], in_=rhs)
        lhs_sb = cpool.tile([19, NSHARD], dt.bfloat16, tag="lhs")
        nc.sync.dma_start(out=lhs_sb[:], in_=lhsT)
        nodex_sb = cpool.tile([128, NTILES * 3], dt.float32, tag="nodex")
        nc.sync.dma_start(out=nodex_sb[:], in_=nodex)
        invb_sb = cpool.tile([128, POOL], dt.float32, tag="invb")
        nc.sync.dma_start(out=invb_sb[:], in_=invbase)
        eye_sb = cpool.tile([128, 128], dt.float32, tag="eye")
        nc.sync.dma_start(out=eye_sb[:], in_=eye)
        wp_sb = cpool.tile([128, 512], dt.float32, tag="wp")
        nc.sync.dma_start(out=wp_sb[:], in_=wp)
        wl1_sb = cpool.tile([128, 512], dt.float32, tag="wl1")
        nc.sync.dma_start(out=wl1_sb[:], in_=wl1)
        wl2_sb = cpool.tile([128, 512], dt.float32, tag="wl2")
        nc.sync.dma_start(out=wl2_sb[:], in_=wl2)
        bproj_sb = cpool.tile([128, 2], dt.float32, tag="bproj")
        nc.sync.dma_start(out=bproj_sb[:], in_=bproj)
        bl1_sb = cpool.tile([128, 2], dt.float32, tag="bl1")
        nc.sync.dma_start(out=bl1_sb[:], in_=bl1)
        bcomb_sb = cpool.tile([128, 2], dt.float32, tag="bcomb")
        nc.sync.dma_start(out=bcomb_sb[:], in_=bcomb)
        wt1_sb = cpool.tile([EMBED, EMBED], dt.float32, tag="wt1")
        nc.sync.dma_start(out=wt1_sb[:], in_=wt1)
        wt2_sb = cpool.tile([EMBED, C], dt.float32, tag="wt2")
        nc.sync.dma_start(out=wt2_sb[:], in_=wt2)
        bt1_sb = cpool.tile([EMBED, 1], dt.float32, tag="bt1")
        nc.sync.dma_start(out=bt1_sb[:], in_=bt1)
        fr_sb = cpool.tile([EMBED, 1], dt.float32, tag="fr")
        nc.sync.dma_start(out=fr_sb[:], in_=freqs)
        sh_sb = cpool.tile([EMBED, 1], dt.float32, tag="sh")
        nc.sync.dma_start(out=sh_sb[:], in_=shifts)
        t_sb = cpool.tile([EMBED, 1], dt.float32, tag="t1x1")
        nc.sync.dma_start(out=t_sb[:], in_=tval)

        # ---- t branch (once) -> fincol [128, 2] = b_l2 + b_t2 + W_t2 @ leaky(W_t1 @ emb + b_t1)
        e = cpool.tile([EMBED, 1], dt.float32, tag="e")
        nc.vector.tensor_mul(e[:], t_sb[:], fr_sb[:])
        # range-reduce e+shift into [-pi, pi]; shift = 0 (sin half) / pi/2 (cos half)
        nc.vector.tensor_add(e[:], e[:], sh_sb[:])
        ki = cpool.tile([EMBED, 1], dt.int32, tag="ki")
        kf = cpool.tile([EMBED, 1], dt.float32, tag="kf")
        nc.vector.tensor_scalar(kf[:], e[:], float(1.0 / (2 * PI)), None, op0=OP.mult)
        nc.vector.tensor_copy(out=ki[:], in_=kf[:])
        nc.vector.tensor_copy(out=kf[:], in_=ki[:])
        nc.vector.tensor_scalar(kf[:], kf[:], float(2 * PI), None, op0=OP.mult)
        nc.vector.tensor_sub(e[:], e[:], kf[:])
        gt = cpool.tile([EMBED, 1], dt.float32, tag="gt")
        nc.vector.tensor_scalar(gt[:], e[:], float(PI), None, op0=OP.is_gt)
        nc.vector.tensor_scalar(gt[:], gt[:], float(2 * PI), None, op0=OP.mult)
        nc.vector.tensor_sub(e[:], e[:], gt[:])
        emb_sb = cpool.tile([EMBED, 1], dt.float32, tag="emb")
        nc.scalar.activation(emb_sb[:], e[:], AF.Sin)
        ps_t1 = ppool.tile([EMBED, 1], dt.float32, tag="mm")
        nc.tensor.matmul(ps_t1[:], lhsT=wt1_sb[:], rhs=emb_sb[:], start=True, stop=True)
        h96 = cpool.tile([EMBED, 1], dt.float32, tag="h96")
        nc.scalar.activation(h96[:], ps_t1[:], AF.Identity, bias=bt1_sb[:, 0:1])
        h96b = cpool.tile([EMBED, 1], dt.float32, tag="h96b")
        nc.vector.tensor_scalar(h96b[:], h96[:], 0.1, None, op0=OP.mult)
        nc.vector.tensor_max(h96b[:], h96b[:], h96[:])
        fincol = cpool.tile([128, 2], dt.float32, tag="fincol")
        for d in range(2):
            ps_t2 = ppool.tile([128, 1], dt.float32, tag="mm")
            nc.tensor.matmul(ps_t2[:], lhsT=wt2_sb[:, d * 128:(d + 1) * 128],
                             rhs=h96b[:], start=True, stop=True)
            nc.scalar.activation(fincol[:, d:d + 1], ps_t2[:], AF.Identity,
                                 bias=bcomb_sb[:, d:d + 1])

        const24 = cpool.tile([128, NCAND], dt.float32, tag="c24")
        nc.vector.memset(const24[:], 32768.0)

        # ---- main loop over node tiles
        for t in range(NTILES):
            lt = lhs_sb[:, t * 128:(t + 1) * 128]
            pool_vals = wpool.tile([128, POOL], dt.float32, tag="pvals")
            pool_lidx = wpool.tile([128, POOL], dt.uint16, tag="plidx")
            for s in range(NSUP):
                ps_d = ppool.tile([128, SUP], dt.float32, tag="dist")
                for h in range(2):
                    nc.tensor.matmul(ps_d[:, h * 512:(h + 1) * 512], lhsT=lt,
                                     rhs=rhs_sb[:, s * SUP + h * 512: s * SUP + (h + 1) * 512],
                                     start=True, stop=True)
                nc.vector.max(out=pool_vals[:, s * 8:(s + 1) * 8], in_=ps_d[:])
                nc.vector.max_index(out=pool_lidx[:, s * 8:(s + 1) * 8],
                                    in_max=pool_vals[:, s * 8:(s + 1) * 8],
                                    in_values=ps_d[:])
                if DEBUG and t == 0 and s == 0:
                    dscratch = wpool.tile([128, SUP], dt.float32, tag="dscr")
                    nc.scalar.activation(dscratch[:], ps_d[:], AF.Identity)
                    nc.sync.dma_start(out=dbg_psum, in_=dscratch[:])
            # pool index decode: inv = invbase - lidx  (inv = 32768 - gidx)
            lidxf = wpool.tile([128, POOL], dt.float32, tag="lidxf")
            nc.vector.tensor_copy(out=lidxf[:], in_=pool_lidx[:])
            pinv = wpool.tile([128, POOL], dt.float32, tag="pinv")
            nc.vector.tensor_sub(pinv[:], invb_sb[:], lidxf[:])
            # top-24 of pool -> mask
            pv2 = wpool.tile([128, POOL], dt.float32, tag="pv2")
            pv3 = wpool.tile([128, POOL], dt.float32, tag="pv3")
            pv4 = wpool.tile([128, POOL], dt.float32, tag="pv4")
            v8 = wpool.tile([128, 8], dt.float32, tag="v8")
            nc.vector.max(out=v8[:], in_=pool_vals[:])
            nc.vector.match_replace(out=pv2[:], in_to_replace=v8[:],
                                    in_values=pool_vals[:], imm_value=-3e38)
            nc.vector.max(out=v8[:], in_=pv2[:])
            nc.vector.match_replace(out=pv3[:], in_to_replace=v8[:],
                                    in_values=pv2[:], imm_value=-3e38)
            nc.vector.max(out=v8[:], in_=pv3[:])
            nc.vector.match_replace(out=pv4[:], in_to_replace=v8[:],
                                    in_values=pv3[:], imm_value=-3e38)
            maskp = wpool.tile([128, POOL], dt.float32, tag="maskp")
            nc.vector.tensor_tensor(out=maskp[:], in0=pv4[:], in1=pool_vals[:],
                                    op=OP.not_equal)
            nc.vector.tensor_mul(maskp[:], maskp[:], pinv[:])
            # extract 24 winner invs (sorted desc = gidx ascending)
            inv24 = wpool.tile([128, NCAND], dt.float32, tag="inv24")
            mv2 = wpool.tile([128, POOL], dt.float32, tag="mv2")
            mv3 = wpool.tile([128, POOL], dt.float32, tag="mv3")
            nc.vector.max(out=inv24[:, 0:8], in_=maskp[:])
            nc.vector.match_replace(out=mv2[:], in_to_replace=inv24[:, 0:8],
                                    in_values=maskp[:], imm_value=0.0)
            nc.vector.max(out=inv24[:, 8:16], in_=mv2[:])
            nc.vector.match_replace(out=mv3[:], in_to_replace=inv24[:, 8:16],
                                    in_values=mv2[:], imm_value=0.0)
            nc.vector.max(out=inv24[:, 16:24], in_=mv3[:])
            # gidx24 = 32768 - inv24 -> uint32
            g24f = wpool.tile([128, NCAND], dt.float32, tag="g24f")
            nc.vector.tensor_sub(g24f[:], const24[:], inv24[:])
            g24u = wpool.tile([128, NCAND], dt.uint32, tag="g24u")
            nc.vector.tensor_copy(out=g24u[:], in_=g24f[:])
            # gather candidate coords [128, 24, 4] (one row per partition per DMA)
            candy = wpool.tile([128, NCAND, 4], dt.float32, tag="candy")
            for k in range(NCAND):
                nc.gpsimd.indirect_dma_start(
                    out=candy[:, k, :], out_offset=None, in_=partc,
                    in_offset=bass.IndirectOffsetOnAxis(ap=g24u[:, k:k + 1], axis=0))

            # ---- exact d2 (Dekker fma chain, bit-exact vs XLA fused body)
            def dekker_d2(ysrc, width, tagp):
                """ysrc: [128, width, 4] gathered coords; returns d2 [128, width]."""
                xs = [nodex_sb[:, t * 3 + k: t * 3 + k + 1] for k in range(3)]
                dcol = [wpool.tile([128, width], dt.float32, tag=f"{tagp}d{k}",
                                   name=f"{tagp}dcol{k}")
                        for k in range(3)]
                for k in range(3):
                    # diff = y - x  (square equals (x-y)^2 bitwise)
                    nc.vector.tensor_scalar(dcol[k][:], ysrc[:, :, k + 1], xs[k],
                                            None, op0=OP.subtract)
                acc = wpool.tile([128, width], dt.float32, tag=f"{tagp}acc")
                nc.vector.tensor_mul(acc[:], dcol[0][:], dcol[0][:])
                tt = [wpool.tile([128, width], dt.float32, tag=f"{tagp}t{i}",
                                 name=f"{tagp}tt{i}")
                      for i in range(6)]
                for k in (1, 2):
                    d = dcol[k]
                    T0, T1, T2, T3, T4, T5 = tt
                    nc.vector.tensor_scalar(T0[:], d[:], 4097.0, None, op0=OP.mult)
                    nc.vector.tensor_sub(T1[:], T0[:], d[:])     # u = t - d
                    nc.vector.tensor_sub(T0[:], T0[:], T1[:])    # xh
                    nc.vector.tensor_sub(T1[:], d[:], T0[:])     # xl
                    nc.vector.tensor_mul(T2[:], d[:], d[:])      # p
                    nc.vector.tensor_mul(T3[:], T0[:], T0[:])    # xh*xh
                    nc.vector.tensor_sub(T3[:], T3[:], T2[:])    # e1 = xh*xh - p
                    nc.vector.tensor_add(T4[:], T1[:], T1[:])    # 2*xl
                    nc.vector.tensor_mul(T4[:], T0[:], T4[:])    # xh*2xl
                    nc.vector.tensor_add(T3[:], T3[:], T4[:])    # e1+e2
                    nc.vector.tensor_mul(T4[:], T1[:], T1[:])    # xl*xl
                    nc.vector.tensor_add(T3[:], T3[:], T4[:])    # e
                    nc.vector.tensor_add(T4[:], T2[:], acc[:])   # s = p + acc
                    nc.vector.tensor_sub(T5[:], T4[:], T2[:])    # t1 = s - p
                    nc.vector.tensor_sub(T0[:], T4[:], T5[:])    # t2 = s - t1
                    nc.vector.tensor_sub(T0[:], T2[:], T0[:])    # t3 = p - t2
                    nc.vector.tensor_sub(T1[:], acc[:], T5[:])   # t4 = acc - t1
                    nc.vector.tensor_add(T0[:], T0[:], T1[:])    # dl = t3+t4
                    nc.vector.tensor_add(T0[:], T0[:], T3[:])    # dl += e
                    nc.vector.tensor_add(acc[:], T4[:], T0[:])   # acc = s + dl
                return acc

            d2c = dekker_d2(candy, NCAND, "c")
            nd2 = wpool.tile([128, NCAND], dt.float32, tag="nd2")
            nc.vector.tensor_scalar(nd2[:], d2c[:], -1.0, None, op0=OP.mult)
            v8f = wpool.tile([128, 8], dt.float32, tag="v8f")
            nc.vector.max(out=v8f[:], in_=nd2[:])
            refb = wpool.tile([128, NCAND], dt.float32, tag="refb")
            nc.vector.match_replace(out=refb[:], in_to_replace=v8f[:],
                                    in_values=nd2[:], imm_value=3e38)
            mask2 = wpool.tile([128, NCAND], dt.float32, tag="mask2")
            nc.vector.tensor_tensor(out=mask2[:], in0=refb[:], in1=nd2[:],
                                    op=OP.not_equal)
            nc.vector.tensor_mul(mask2[:], mask2[:], inv24[:])
            inv8 = wpool.tile([128, 8], dt.float32, tag="inv8")
            nc.vector.max(out=inv8[:], in_=mask2[:])
            g8f = wpool.tile([128, 8], dt.float32, tag="g8f")
            nc.vector.tensor_sub(g8f[:], const24[:, 0:8], inv8[:])
            g8u = wpool.tile([128, 8], dt.uint32, tag="g8u")
            nc.vector.tensor_copy(out=g8u[:], in_=g8f[:])

            # winners' coords + exact d2 -> weights
            winy = wpool.tile([128, 8, 4], dt.float32, tag="winy")
            for k in range(8):
                nc.gpsimd.indirect_dma_start(
                    out=winy[:, k, :], out_offset=None, in_=partc,
                    in_offset=bass.IndirectOffsetOnAxis(ap=g8u[:, k:k + 1], axis=0))
            d28 = dekker_d2(winy, 8, "w")
            dist = wpool.tile([128, 8], dt.float32, tag="dist8")
            nc.scalar.activation(dist[:], d28[:], AF.Sqrt)
            nc.vector.tensor_scalar_max(dist[:], dist[:], 1e-6)
            wr = wpool.tile([128, 8], dt.float32, tag="wr")
            nc.vector.reciprocal(out=wr[:], in_=dist[:])
            wsum = wpool.tile([128, 1], dt.float32, tag="wsum")
            nc.vector.tensor_reduce(out=wsum[:], in_=wr[:],
                                    axis=mybir.AxisListType.X, op=OP.add)
            wsr = wpool.tile([128, 1], dt.float32, tag="wsr")
            nc.vector.reciprocal(out=wsr[:], in_=wsum[:])
            wn = wpool.tile([128, 8], dt.float32, tag="wn")
            nc.vector.tensor_scalar(wn[:], wr[:], wsr[:, 0:1], None, op0=OP.mult)

            # gather feats [128, 8, 256] and weighted-sum on gpsimd
            gf = wpool.tile([128, 8, C], dt.float32, tag="gf")
            for k in range(8):
                nc.gpsimd.indirect_dma_start(
                    out=gf[:, k, :], out_offset=None, in_=feats,
                    in_offset=bass.IndirectOffsetOnAxis(ap=g8u[:, k:k + 1], axis=0))
            fbar = wpool.tile([128, C], dt.float32, tag="fbar")
            nc.vector.tensor_scalar(fbar[:], gf[:, 0, :], wn[:, 0:1], None,
                                    op0=OP.mult)
            for k in range(1, 8):
                nc.vector.scalar_tensor_tensor(out=fbar[:], in0=gf[:, k, :],
                                               scalar=wn[:, k:k + 1], in1=fbar[:],
                                               op0=OP.mult, op1=OP.add)
            if DEBUG and t == 0:
                nc.sync.dma_start(out=dbg_pv, in_=pool_vals[:])
                nc.sync.dma_start(out=dbg_pl, in_=pool_lidx[:])
                nc.sync.dma_start(out=dbg_inv, in_=inv24[:])
                nc.sync.dma_start(out=dbg_g24, in_=g24u[:])
                nc.sync.dma_start(out=dbg_d2, in_=d2c[:])
                nc.sync.dma_start(out=dbg_candy, in_=candy[:])
                nc.sync.dma_start(out=dbg_wn, in_=wn[:])
                nc.sync.dma_start(out=dbg_fbar, in_=fbar[:])
                nc.sync.dma_start(out=dbg_g8, in_=g8u[:])
                nc.sync.dma_start(out=dbg_winy, in_=winy[:])

            # transpose fbar -> fT [128c, 256] (two c-tiles of nodes)
            fT = wpool.tile([128, C], dt.float32, tag="fT")
            for cth in range(2):
                ps_tr = ppool.tile([128, 128], dt.float32, tag="tr")
                nc.tensor.transpose(ps_tr[:], fbar[:, cth * 128:(cth + 1) * 128],
                                    eye_sb[:])
                nc.scalar.activation(fT[:, cth * 128:(cth + 1) * 128], ps_tr[:],
                                     AF.Identity)

            # MLP in transposed space
            def dense(src_sb, wpack, bias_sb, leaky, outtag):
                o = wpool.tile([128, C], dt.float32, tag=outtag)
                for d in range(2):
                    ps = ppool.tile([128, 128], dt.float32, tag="mm")
                    for ct in range(2):
                        nc.tensor.matmul(
                            ps[:], lhsT=wpack[:, (ct * 2 + d) * 128:(ct * 2 + d + 1) * 128],
                            rhs=src_sb[:, ct * 128:(ct + 1) * 128],
                            start=(ct == 0), stop=(ct == 1))
                    nc.scalar.activation(o[:, d * 128:(d + 1) * 128], ps[:],
                                         AF.Identity, bias=bias_sb[:, d:d + 1])
                if leaky:
                    tmp = wpool.tile([128, C], dt.float32, tag=outtag + "lk")
                    nc.vector.tensor_scalar(tmp[:], o[:], 0.1, None, op0=OP.mult)
                    nc.vector.tensor_max(o[:], o[:], tmp[:])
                return o

            mT = dense(fT, wp_sb, bproj_sb, False, "mT")
            h1T = dense(mT, wl1_sb, bl1_sb, True, "h1T")
            pT = dense(h1T, wl2_sb, fincol, False, "pT")
            if DEBUG and t == 0:
                nc.sync.dma_start(out=dbg_fT, in_=fT[:])
                nc.sync.dma_start(out=dbg_mT, in_=mT[:])
                nc.sync.dma_start(out=dbg_fin, in_=fincol[:])
                nc.sync.dma_start(out=dbg_h1, in_=h1T[:])
                nc.sync.dma_start(out=dbg_pT, in_=pT[:])

            # transpose back to node-major and store
            osb = wpool.tile([128, C], dt.float32, tag="osb")
            for dth in range(2):
                ps_tr = ppool.tile([128, 128], dt.float32, tag="tr")
                nc.tensor.transpose(ps_tr[:], pT[:, dth * 128:(dth + 1) * 128],
                                    eye_sb[:])
                nc.scalar.activation(osb[:, dth * 128:(dth + 1) * 128], ps_tr[:],
                                     AF.Identity)
            if DEBUG and t == 0:
                nc.sync.dma_start(out=dbg_osb, in_=osb[:])
            nc.sync.dma_start(out=out[t * 128:(t + 1) * 128, :], in_=osb[:])

    nc.compile()
    names = [n for n in T]
    _CACHE['nc'] = nc
    _CACHE['names'] = names
    return nc, names


# ---------------------------------------------------------------- host entry
def kernel(node_coords, cond_coords, cond_feats, t,
           W_proj, b_proj, W_l1, b_l1, W_l2, b_l2, W_t1, b_t1, W_t2, b_t2):
    node_coords = np.asarray(node_coords)
    cond_coords = np.asarray(cond_coords)
    cond_feats = np.ascontiguousarray(np.asarray(cond_feats, dtype=np.float32))
    mc = np.float32(node_coords.astype(np.float32).max())
    part_c = np.ascontiguousarray(_transform(cond_coords, 1.0, 0.01, mc))
    LHS, RHS = _split_rows(node_coords, cond_coords)
    full_c = _transform(node_coords, 16.0, 0.05, mc)

    invbase = np.tile((32768.0 - (np.arange(POOL) // 8) * SUP).astype(f32)[None, :],
                      (128, 1))
    eye = np.eye(128, dtype=f32)
    half = HALF
    freqs1 = np.exp(np.arange(half, dtype=np.float32) *
                    f32(-math.log(10000.0) / (half - 1))).astype(f32)
    freqs = np.concatenate([freqs1, freqs1])
    shifts = np.concatenate([np.zeros(half, f32),
                             np.full(half, PI / 2, f32)])

    nc, _ = _build_program()
    in_maps = []
    for i in range(NCORES):
        sl = slice(i * NSHARD, (i + 1) * NSHARD)
        nodex = np.ascontiguousarray(
            full_c[sl, 1:4].reshape(NTILES, 128, 3).transpose(1, 0, 2)
            .reshape(128, NTILES * 3))
        in_maps.append({
            'lhsT': np.ascontiguousarray(LHS[:, sl]),
            'rhsT': RHS,
            'nodex': nodex,
            'partc': part_c,
            'feats': cond_feats,
            'invbase': invbase,
            'eye': eye,
            'wp': _pack_w(np.asarray(W_proj, dtype=f32)),
            'wl1': _pack_w(np.asarray(W_l1, dtype=f32)),
            'wl2': _pack_w(np.asarray(W_l2, dtype=f32)),
            'bproj': np.asarray(b_proj, f32).reshape(2, 128).T.copy(),
            'bl1': np.asarray(b_l1, f32).reshape(2, 128).T.copy(),
            'bcomb': (np.asarray(b_l2, f32) + np.asarray(b_t2, f32)).reshape(2, 128).T.copy(),
            'wt1': np.ascontiguousarray(np.asarray(W_t1, f32).T),
            'wt2': np.ascontiguousarray(np.asarray(W_t2, f32).T),
            'bt1': np.asarray(b_t1, f32).reshape(EMBED, 1).copy(),
            'freqs': freqs.reshape(EMBED, 1).copy(),
            'shifts': shifts.reshape(EMBED, 1).copy(),
            'tval': np.full((EMBED, 1), np.asarray(t, f32).reshape(()), f32),
        })
    res = bass_utils.run_bass_kernel_spmd(nc, in_maps, core_ids=list(range(NCORES)))
    _CACHE['last_result'] = res
    outs = [res.results[i]['out'] for i in range(NCORES)]
    return np.concatenate(outs, 0)


# revision 40
# speedup vs baseline: 1.3750x; 1.3750x over previous
"""Trainium2 Bass kernel for AttentiveMinkUNetDiff KNN+MLP block (v2).

Self-contained: hardcodes shapes N=16384, M=32768, K=8, C=256, 8 cores.
Sharding: nodes across 8 cores; cond set replicated.

Per core (2048 nodes, 16 tiles of 128):
  1. PE: exact bf16-split integer matmul (K=19 rows) producing a
     per-node-ranking-equivalent of -40000*d^2 for all 32768 cond points.
  2. ACT copies PSUM->SBUF row buffers; DVE max/max_index per 8192-wide
     super -> 32-candidate pool with within-super indices.
  3. Top-16 of pool by value (match_replace rounds), then re-sorted by
     ascending cond index (jax.lax.top_k tie order) via masked-max trick.
  4. One packed indirect-DMA gather per candidate ([coords|feats] rows);
     exact d^2 recomputed bit-exactly vs XLA's fused fma chain (Dekker).
  5. Final 8 by exact value; inverse-distance weights; weighted mean of
     feats via ACT scaling + PE transpose-accumulate (weights sum to 1 so
     the mean commutes with W_proj); 3-layer MLP in transposed space;
     timestep-embedding branch folded into the final bias.
"""
import math
import numpy as np
import ml_dtypes

import concourse.bass as bass
import concourse.mybir as mybir
from concourse.tile import TileContext
from concourse import bass_utils
from concourse import bacc

bf16 = ml_dtypes.bfloat16
f32 = np.float32
AF = mybir.ActivationFunctionType
OP = mybir.AluOpType

N, M, K = 16384, 32768, 8
C = 256
PACK = C + 4                  # packed row: [part_c(4) | feats(256)]
EMBED, HALF = 96, 48
NCORES = 8
NSHARD = N // NCORES          # 2048
NTILES = NSHARD // 128        # 16
SUP = 8192                    # super-chunk width scanned from SBUF
NSUP = M // SUP               # 4
POOL = NSUP * 8               # 32
NCAND = 16
PI = float(np.pi)


# ---------------------------------------------------------------- host prep
def _split_rows(nodes, conds):
    """Build the 19 bf16-exact contraction rows. Validated vs reference."""
    a = nodes[:, 1:4].astype(np.int64)
    b = conds[:, 1:4].astype(np.int64)
    ah, al = a >> 5, a & 31
    bh, bl = b >> 5, b & 31
    lhs, rhs = [], []
    for k in range(3):
        lhs += [1280.0 * ah[:, k], 1280.0 * ah[:, k], 40.0 * al[:, k], 40.0 * al[:, k]]
        rhs += [32.0 * bh[:, k], 1.0 * bl[:, k], 32.0 * bh[:, k], 1.0 * bl[:, k]]
    B_total = (4 * b * b - 316 * b).sum(1) + 32768
    s2, s1, s0 = B_total >> 16, (B_total >> 8) & 255, B_total & 255
    nones = -np.ones(a.shape[0])
    lhs += [nones, nones, nones]
    rhs += [s2 * 65536.0, s1 * 256.0, s0 * 1.0]
    C_i = ((10 * a + 79) ** 2).sum(1) - 32768
    c3 = np.floor(C_i / 2 ** 21).astype(np.int64)
    r = C_i - c3 * 2 ** 21
    c2, c1, c0 = r >> 13, (r >> 5) & 255, r & 31
    mones = np.ones(b.shape[0])
    lhs += [-c3 * 2097152.0, -c2 * 8192.0, -c1 * 32.0, -c0 * 1.0]
    rhs += [mones, mones, mones, mones]
    LHS = np.stack(lhs).astype(f32)   # [19, N]
    RHS = np.stack(rhs).astype(f32)   # [19, M]
    return LHS.astype(bf16), RHS.astype(bf16)


def _transform(coords, stride, voxel, mc):
    c = coords.astype(np.float32)
    batch = (c[:, :1] * f32(mc * f32(2.0))).astype(f32)
    xyz = ((c[:, 1:] + f32(stride / 2.0)).astype(f32) * f32(voxel)).astype(f32)
    return np.concatenate([batch, xyz], 1).astype(f32)


def _pack_w(w):
    """W [dout, din] -> lhsT pack [128, 4*128]: col block (ct*2+dt)."""
    wt = np.ascontiguousarray(w.T.astype(f32))          # [din, dout]
    p = wt.reshape(2, 128, 2, 128)                      # [ct, c, dt, d]
    p = p.transpose(1, 0, 2, 3).reshape(128, 512)
    return np.ascontiguousarray(p)


_CACHE = {}


def _build_program():
    if 'nc' in _CACHE:
        return _CACHE['nc']
    nc = bacc.Bacc("TRN2", target_bir_lowering=False, debug=False,
                   num_devices=NCORES)
    dt = mybir.dt

    def din(name, shape, dtype):
        return nc.dram_tensor(name, shape, dtype, kind="ExternalInput").ap()

    lhsT = din('lhsT', [19, NSHARD], dt.bfloat16)
    rhs = din('rhsT', [19, M], dt.bfloat16)
    nodex = din('nodex', [128, NTILES * 3], dt.float32)
    packed = din('packed', [M, PACK], dt.float32)
    invbase = din('invbase', [128, POOL], dt.float32)
    eye = din('eye', [128, 128], dt.float32)
    wp = din('wp', [128, 512], dt.float32)
    wl1 = din('wl1', [128, 512], dt.float32)
    wl2 = din('wl2', [128, 512], dt.float32)
    bproj = din('bproj', [128, 2], dt.float32)
    bl1 = din('bl1', [128, 2], dt.float32)
    bcomb = din('bcomb', [128, 2], dt.float32)
    wt1 = din('wt1', [EMBED, EMBED], dt.float32)
    wt2 = din('wt2', [EMBED, C], dt.float32)
    bt1 = din('bt1', [EMBED, 1], dt.float32)
    freqs = din('freqs', [EMBED, 1], dt.float32)
    shifts = din('shifts', [EMBED, 1], dt.float32)
    tval = din('tval', [EMBED, 1], dt.float32)
    out = nc.dram_tensor('out', [NSHARD, C], dt.float32, kind="ExternalOutput").ap()

    with TileContext(nc) as tc, \
            tc.tile_pool(name="const", bufs=1) as cpool, \
            tc.tile_pool(name="work", bufs=2) as wpool, \
            tc.tile_pool(name="psum", bufs=2, space="PSUM") as ppool:

        # ---- constants to SBUF
        rhs_sb = cpool.tile([19, M], dt.bfloat16, tag="rhs")
        nc.sync.dma_start(out=rhs_sb[:], in_=rhs)
        lhs_sb = cpool.tile([19, NSHARD], dt.bfloat16, tag="lhs")
        nc.sync.dma_start(out=lhs_sb[:], in_=lhsT)
        nodex_sb = cpool.tile([128, NTILES * 3], dt.float32, tag="nodex")
        nc.sync.dma_start(out=nodex_sb[:], in_=nodex)
        invb_sb = cpool.tile([128, POOL], dt.float32, tag="invb")
        nc.sync.dma_start(out=invb_sb[:], in_=invbase)
        eye_sb = cpool.tile([128, 128], dt.float32, tag="eye")
        nc.sync.dma_start(out=eye_sb[:], in_=eye)
        wp_sb = cpool.tile([128, 512], dt.float32, tag="wp")
        nc.sync.dma_start(out=wp_sb[:], in_=wp)
        wl1_sb = cpool.tile([128, 512], dt.float32, tag="wl1")
        nc.sync.dma_start(out=wl1_sb[:], in_=wl1)
        wl2_sb = cpool.tile([128, 512], dt.float32, tag="wl2")
        nc.sync.dma_start(out=wl2_sb[:], in_=wl2)
        bproj_sb = cpool.tile([128, 2], dt.float32, tag="bproj")
        nc.sync.dma_start(out=bproj_sb[:], in_=bproj)
        bl1_sb = cpool.tile([128, 2], dt.float32, tag="bl1")
        nc.sync.dma_start(out=bl1_sb[:], in_=bl1)
        bcomb_sb = cpool.tile([128, 2], dt.float32, tag="bcomb")
        nc.sync.dma_start(out=bcomb_sb[:], in_=bcomb)
        wt1_sb = cpool.tile([EMBED, EMBED], dt.float32, tag="wt1")
        nc.sync.dma_start(out=wt1_sb[:], in_=wt1)
        wt2_sb = cpool.tile([EMBED, C], dt.float32, tag="wt2")
        nc.sync.dma_start(out=wt2_sb[:], in_=wt2)
        bt1_sb = cpool.tile([EMBED, 1], dt.float32, tag="bt1")
        nc.sync.dma_start(out=bt1_sb[:], in_=bt1)
        fr_sb = cpool.tile([EMBED, 1], dt.float32, tag="fr")
        nc.sync.dma_start(out=fr_sb[:], in_=freqs)
        sh_sb = cpool.tile([EMBED, 1], dt.float32, tag="sh")
        nc.sync.dma_start(out=sh_sb[:], in_=shifts)
        t_sb = cpool.tile([EMBED, 1], dt.float32, tag="t1x1")
        nc.sync.dma_start(out=t_sb[:], in_=tval)

        # ---- t branch -> fincol [128, 2]
        e = cpool.tile([EMBED, 1], dt.float32, tag="e")
        nc.vector.tensor_mul(e[:], t_sb[:], fr_sb[:])
        nc.vector.tensor_add(e[:], e[:], sh_sb[:])
        ki = cpool.tile([EMBED, 1], dt.int32, tag="ki")
        kf = cpool.tile([EMBED, 1], dt.float32, tag="kf")
        nc.vector.tensor_scalar(kf[:], e[:], float(1.0 / (2 * PI)), None, op0=OP.mult)
        nc.vector.tensor_copy(out=ki[:], in_=kf[:])
        nc.vector.tensor_copy(out=kf[:], in_=ki[:])
        nc.vector.tensor_scalar(kf[:], kf[:], float(2 * PI), None, op0=OP.mult)
        nc.vector.tensor_sub(e[:], e[:], kf[:])
        gt = cpool.tile([EMBED, 1], dt.float32, tag="gt")
        nc.vector.tensor_scalar(gt[:], e[:], float(PI), None, op0=OP.is_gt)
        nc.vector.tensor_scalar(gt[:], gt[:], float(2 * PI), None, op0=OP.mult)
        nc.vector.tensor_sub(e[:], e[:], gt[:])
        emb_sb = cpool.tile([EMBED, 1], dt.float32, tag="emb")
        nc.scalar.activation(emb_sb[:], e[:], AF.Sin)
        ps_t1 = ppool.tile([EMBED, 1], dt.float32, tag="mm")
        nc.tensor.matmul(ps_t1[:], lhsT=wt1_sb[:], rhs=emb_sb[:], start=True, stop=True)
        h96 = cpool.tile([EMBED, 1], dt.float32, tag="h96")
        nc.scalar.activation(h96[:], ps_t1[:], AF.Identity, bias=bt1_sb[:, 0:1])
        h96b = cpool.tile([EMBED, 1], dt.float32, tag="h96b")
        nc.vector.tensor_scalar(h96b[:], h96[:], 0.1, None, op0=OP.mult)
        nc.vector.tensor_max(h96b[:], h96b[:], h96[:])
        fincol = cpool.tile([128, 2], dt.float32, tag="fincol")
        for d in range(2):
            ps_t2 = ppool.tile([128, 1], dt.float32, tag="mm")
            nc.tensor.matmul(ps_t2[:], lhsT=wt2_sb[:, d * 128:(d + 1) * 128],
                             rhs=h96b[:], start=True, stop=True)
            nc.scalar.activation(fincol[:, d:d + 1], ps_t2[:], AF.Identity,
                                 bias=bcomb_sb[:, d:d + 1])

        c32k = cpool.tile([128, NCAND], dt.float32, tag="c32k")
        nc.vector.memset(c32k[:], 32768.0)

        # ---- main loop over node tiles
        for t in range(NTILES):
            lt = lhs_sb[:, t * 128:(t + 1) * 128]
            pool_vals = wpool.tile([128, POOL], dt.float32, tag="pvals")
            pool_lidx = wpool.tile([128, POOL], dt.uint16, tag="plidx")
            for s in range(NSUP):
                rowb = wpool.tile([128, SUP], dt.float32, tag="rowb")
                for h in range(SUP // 1024):
                    ps_d = ppool.tile([128, 1024], dt.float32, tag="dist")
                    for q in range(2):
                        c0 = s * SUP + h * 1024 + q * 512
                        nc.tensor.matmul(ps_d[:, q * 512:(q + 1) * 512], lhsT=lt,
                                         rhs=rhs_sb[:, c0:c0 + 512],
                                         start=True, stop=True)
                    nc.scalar.activation(rowb[:, h * 1024:(h + 1) * 1024], ps_d[:],
                                         AF.Identity)
                nc.vector.max(out=pool_vals[:, s * 8:(s + 1) * 8], in_=rowb[:])
                nc.vector.max_index(out=pool_lidx[:, s * 8:(s + 1) * 8],
                                    in_max=pool_vals[:, s * 8:(s + 1) * 8],
                                    in_values=rowb[:])
            # pool: inv = invbase - lidx
            lidxf = wpool.tile([128, POOL], dt.float32, tag="lidxf")
            nc.vector.tensor_copy(out=lidxf[:], in_=pool_lidx[:])
            pinv = wpool.tile([128, POOL], dt.float32, tag="pinv")
            nc.vector.tensor_sub(pinv[:], invb_sb[:], lidxf[:])
            # top-16 by value -> mask
            pv2 = wpool.tile([128, POOL], dt.float32, tag="pv2")
            pv3 = wpool.tile([128, POOL], dt.float32, tag="pv3")
            v8 = wpool.tile([128, 8], dt.float32, tag="v8")
            nc.vector.max(out=v8[:], in_=pool_vals[:])
            nc.vector.match_replace(out=pv2[:], in_to_replace=v8[:],
                                    in_values=pool_vals[:], imm_value=-3e38)
            nc.vector.max(out=v8[:], in_=pv2[:])
            nc.vector.match_replace(out=pv3[:], in_to_replace=v8[:],
                                    in_values=pv2[:], imm_value=-3e38)
            maskp = wpool.tile([128, POOL], dt.float32, tag="maskp")
            nc.vector.tensor_tensor(out=maskp[:], in0=pv3[:], in1=pool_vals[:],
                                    op=OP.not_equal)
            nc.vector.tensor_mul(maskp[:], maskp[:], pinv[:])
            # extract 16 winner invs sorted desc (= gidx ascending)
            inv16 = wpool.tile([128, NCAND], dt.float32, tag="inv16")
            mv2 = wpool.tile([128, POOL], dt.float32, tag="mv2")
            nc.vector.max(out=inv16[:, 0:8], in_=maskp[:])
            nc.vector.match_replace(out=mv2[:], in_to_replace=inv16[:, 0:8],
                                    in_values=maskp[:], imm_value=0.0)
            nc.vector.max(out=inv16[:, 8:16], in_=mv2[:])
            g16f = wpool.tile([128, NCAND], dt.float32, tag="g16f")
            nc.vector.tensor_sub(g16f[:], c32k[:], inv16[:])
            g16u = wpool.tile([128, NCAND], dt.uint32, tag="g16u")
            nc.vector.tensor_copy(out=g16u[:], in_=g16f[:])
            # packed gather: [coords|feats] per candidate
            gp = wpool.tile([128, NCAND, PACK], dt.float32, tag="gp")
            for k in range(NCAND):
                nc.gpsimd.indirect_dma_start(
                    out=gp[:, k, :], out_offset=None, in_=packed,
                    in_offset=bass.IndirectOffsetOnAxis(ap=g16u[:, k:k + 1], axis=0))

            # ---- exact d2 (Dekker fma chain)
            xs = [nodex_sb[:, t * 3 + k: t * 3 + k + 1] for k in range(3)]
            dcol = [wpool.tile([128, NCAND], dt.float32, tag=f"d{k}",
                               name=f"dcol{k}") for k in range(3)]
            for k in range(3):
                nc.vector.tensor_scalar(dcol[k][:], gp[:, :, k + 1], xs[k],
                                        None, op0=OP.subtract)
            acc = wpool.tile([128, NCAND], dt.float32, tag="acc")
            nc.vector.tensor_mul(acc[:], dcol[0][:], dcol[0][:])
            tt = [wpool.tile([128, NCAND], dt.float32, tag=f"t{i}",
                             name=f"tt{i}") for i in range(6)]
            for k in (1, 2):
                d = dcol[k]
                T0, T1, T2, T3, T4, T5 = tt
                nc.vector.tensor_scalar(T0[:], d[:], 4097.0, None, op0=OP.mult)
                nc.vector.tensor_sub(T1[:], T0[:], d[:])
                nc.vector.tensor_sub(T0[:], T0[:], T1[:])    # xh
                nc.vector.tensor_sub(T1[:], d[:], T0[:])     # xl
                nc.vector.tensor_mul(T2[:], d[:], d[:])      # p
                nc.vector.tensor_mul(T3[:], T0[:], T0[:])
                nc.vector.tensor_sub(T3[:], T3[:], T2[:])
                nc.vector.tensor_add(T4[:], T1[:], T1[:])
                nc.vector.tensor_mul(T4[:], T0[:], T4[:])
                nc.vector.tensor_add(T3[:], T3[:], T4[:])
                nc.vector.tensor_mul(T4[:], T1[:], T1[:])
                nc.vector.tensor_add(T3[:], T3[:], T4[:])    # e
                nc.vector.tensor_add(T4[:], T2[:], acc[:])   # s
                nc.vector.tensor_sub(T5[:], T4[:], T2[:])
                nc.vector.tensor_sub(T0[:], T4[:], T5[:])
                nc.vector.tensor_sub(T0[:], T2[:], T0[:])
                nc.vector.tensor_sub(T1[:], acc[:], T5[:])
                nc.vector.tensor_add(T0[:], T0[:], T1[:])
                nc.vector.tensor_add(T0[:], T0[:], T3[:])
                nc.vector.tensor_add(acc[:], T4[:], T0[:])
            # final 8 by exact value (ties -> lowest position = lowest gidx)
            nd2 = wpool.tile([128, NCAND], dt.float32, tag="nd2")
            nc.vector.tensor_scalar(nd2[:], acc[:], -1.0, None, op0=OP.mult)
            v8f = wpool.tile([128, 8], dt.float32, tag="v8f")
            nc.vector.max(out=v8f[:], in_=nd2[:])
            refb = wpool.tile([128, NCAND], dt.float32, tag="refb")
            nc.vector.match_replace(out=refb[:], in_to_replace=v8f[:],
                                    in_values=nd2[:], imm_value=3e38)
            mask2 = wpool.tile([128, NCAND], dt.float32, tag="mask2")
            nc.vector.tensor_tensor(out=mask2[:], in0=refb[:], in1=nd2[:],
                                    op=OP.not_equal)
            # weights over the 16 slots (non-winners get 0)
            dist = wpool.tile([128, NCAND], dt.float32, tag="dist16")
            nc.scalar.activation(dist[:], acc[:], AF.Sqrt)
            nc.vector.tensor_scalar_max(dist[:], dist[:], 1e-6)
            wr = wpool.tile([128, NCAND], dt.float32, tag="wr")
            nc.vector.reciprocal(out=wr[:], in_=dist[:])
            nc.vector.tensor_mul(wr[:], wr[:], mask2[:])
            wsum = wpool.tile([128, 1], dt.float32, tag="wsum")
            nc.vector.tensor_reduce(out=wsum[:], in_=wr[:],
                                    axis=mybir.AxisListType.X, op=OP.add)
            wsr = wpool.tile([128, 1], dt.float32, tag="wsr")
            nc.vector.reciprocal(out=wsr[:], in_=wsum[:])
            wn = wpool.tile([128, NCAND], dt.float32, tag="wn")
            nc.vector.tensor_scalar(wn[:], wr[:], wsr[:, 0:1], None, op0=OP.mult)

            # weighted mean of feats -> fbar^T via ACT scale + PE transpose-acc
            ps_fT = ppool.tile([128, C], dt.float32, tag="tr")
            for k in range(NCAND):
                sc = wpool.tile([128, C], dt.float32, tag="sc", bufs=3)
                nc.scalar.activation(sc[:], gp[:, k, 4:4 + C], AF.Identity,
                                     scale=wn[:, k:k + 1])
                for half_i in range(2):
                    nc.tensor.matmul(
                        ps_fT[:, half_i * 128:(half_i + 1) * 128],
                        lhsT=sc[:, half_i * 128:(half_i + 1) * 128],
                        rhs=eye_sb[:], is_transpose=True,
                        start=(k == 0 and half_i == 0),
                        stop=(k == NCAND - 1 and half_i == 1))
            fT = wpool.tile([128, C], dt.float32, tag="fT")
            nc.scalar.activation(fT[:], ps_fT[:], AF.Identity)

            # ---- MLP in transposed space
            def dense(src_sb, wpack, bias_sb, leaky, outtag):
                o = wpool.tile([128, C], dt.float32, tag=outtag, name=outtag)
                for d in range(2):
                    ps = ppool.tile([128, 128], dt.float32, tag="mm", name="psmm")
                    for ct in range(2):
                        nc.tensor.matmul(
                            ps[:], lhsT=wpack[:, (ct * 2 + d) * 128:(ct * 2 + d + 1) * 128],
                            rhs=src_sb[:, ct * 128:(ct + 1) * 128],
                            start=(ct == 0), stop=(ct == 1))
                    nc.scalar.activation(o[:, d * 128:(d + 1) * 128], ps[:],
                                         AF.Identity, bias=bias_sb[:, d:d + 1])
                if leaky:
                    tmp = wpool.tile([128, C], dt.float32, tag=outtag + "lk",
                                     name=outtag + "lk")
                    nc.vector.tensor_scalar(tmp[:], o[:], 0.1, None, op0=OP.mult)
                    nc.vector.tensor_max(o[:], o[:], tmp[:])
                return o

            mT = dense(fT, wp_sb, bproj_sb, False, "mT")
            h1T = dense(mT, wl1_sb, bl1_sb, True, "h1T")
            pT = dense(h1T, wl2_sb, fincol, False, "pT")

            osb = wpool.tile([128, C], dt.float32, tag="osb")
            for dth in range(2):
                ps_tr = ppool.tile([128, 128], dt.float32, tag="mm", name="pstr")
                nc.tensor.matmul(ps_tr[:], lhsT=pT[:, dth * 128:(dth + 1) * 128],
                                 rhs=eye_sb[:], is_transpose=True,
                                 start=True, stop=True)
                nc.scalar.activation(osb[:, dth * 128:(dth + 1) * 128], ps_tr[:],
                                     AF.Identity)
            nc.sync.dma_start(out=out[t * 128:(t + 1) * 128, :], in_=osb[:])

    nc.compile()
    _CACHE['nc'] = nc
    return nc


# ---------------------------------------------------------------- host entry
def kernel(node_coords, cond_coords, cond_feats, t,
           W_proj, b_proj, W_l1, b_l1, W_l2, b_l2, W_t1, b_t1, W_t2, b_t2):
    node_coords = np.asarray(node_coords)
    cond_coords = np.asarray(cond_coords)
    cond_feats = np.asarray(cond_feats, dtype=np.float32)
    mc = np.float32(node_coords.astype(np.float32).max())
    part_c = _transform(cond_coords, 1.0, 0.01, mc)
    packed = np.ascontiguousarray(np.concatenate([part_c, cond_feats], 1))
    LHS, RHS = _split_rows(node_coords, cond_coords)
    full_c = _transform(node_coords, 16.0, 0.05, mc)

    invbase = np.tile((32768.0 - (np.arange(POOL) // 8) * SUP).astype(f32)[None, :],
                      (128, 1))
    eye = np.eye(128, dtype=f32)
    freqs1 = np.exp(np.arange(HALF, dtype=np.float32) *
                    f32(-math.log(10000.0) / (HALF - 1))).astype(f32)
    freqs = np.concatenate([freqs1, freqs1])
    shifts = np.concatenate([np.zeros(HALF, f32), np.full(HALF, PI / 2, f32)])

    nc = _build_program()
    in_maps = []
    for i in range(NCORES):
        sl = slice(i * NSHARD, (i + 1) * NSHARD)
        nodex = np.ascontiguousarray(
            full_c[sl, 1:4].reshape(NTILES, 128, 3).transpose(1, 0, 2)
            .reshape(128, NTILES * 3))
        in_maps.append({
            'lhsT': np.ascontiguousarray(LHS[:, sl]),
            'rhsT': RHS,
            'nodex': nodex,
            'packed': packed,
            'invbase': invbase,
            'eye': eye,
            'wp': _pack_w(np.asarray(W_proj, dtype=f32)),
            'wl1': _pack_w(np.asarray(W_l1, dtype=f32)),
            'wl2': _pack_w(np.asarray(W_l2, dtype=f32)),
            'bproj': np.asarray(b_proj, f32).reshape(2, 128).T.copy(),
            'bl1': np.asarray(b_l1, f32).reshape(2, 128).T.copy(),
            'bcomb': (np.asarray(b_l2, f32) + np.asarray(b_t2, f32)).reshape(2, 128).T.copy(),
            'wt1': np.ascontiguousarray(np.asarray(W_t1, f32).T),
            'wt2': np.ascontiguousarray(np.asarray(W_t2, f32).T),
            'bt1': np.asarray(b_t1, f32).reshape(EMBED, 1).copy(),
            'freqs': freqs.reshape(EMBED, 1).copy(),
            'shifts': shifts.reshape(EMBED, 1).copy(),
            'tval': np.full((EMBED, 1), np.asarray(t, f32).reshape(()), f32),
        })
    res = bass_utils.run_bass_kernel_spmd(nc, in_maps, core_ids=list(range(NCORES)))
    _CACHE['last_result'] = res
    outs = [res.results[i]['out'] for i in range(NCORES)]
    return np.concatenate(outs, 0)
